# revision 8
# baseline (speedup 1.0000x reference)
"""Trainium2 Bass kernel for the non-local attention block (dense_transformer).

Reference computation per batch item b (x: [B=32, C=64, H=32, W=32], N=1024):
    xf    = x[b] reshaped [C, N]
    phi   = w_phi   @ xf                     [C, N]
    theta = (w_theta @ xf)^T                 [N, C]
    g     = (w_g @ xf)^T @ w_mv^T            [N, C]
    att   = theta @ phi                      [N, N]
    att   = att @ w_mk^T                     [N, N]
    att   = softmax(att, axis over rows n)
    out   = att @ g                          [N, C]
    final = w_mask @ out^T + xf              [C, N]

Key algebraic restructure: (theta @ phi) @ w_mk^T == theta @ (phi @ w_mk^T),
which removes the N^3 matmul (1073M MACs -> 2x67M MACs per batch).  The
softmax denominator divide is folded into the small g factor (64 wide)
instead of the [N, N] attention matrix.

Per-core layout (data-parallel, 4 batch items per core, processed as 2
stacked pairs occupying the 128 SBUF partitions; batch "b" on partitions
0-63, batch "c" on 64-127, PE quadrant tile-position packing runs both
batches' matmuls concurrently):
    T    = w_theta @ xf          [64, 1024]  (diag-quadrant pair matmuls)
    PhiT = xf^T @ w_phi^T        [1024, 64]  (row-split pair matmuls)
    GT   = xf^T @ (w_mv@w_g)^T   [1024, 64]  (row-split)
    P2   = PhiT^T @ w_mk^T       [64, 1024]  (col-split, accum over 8 m-chunks)
    S    = P2^T @ T              [1024, 1024] = att2^T  (row-split per k-chunk)
    E    = exp(S)  (ScalarE, fused row-sum via accum_out -> D)
    GTs  = GT * (1/D)            (fold softmax divide into g)
    O    = GTs^T @ E             [64, 1024]  (col-split, accum over m-chunks)
    final= w_mask @ O + xf       (diag-quadrant + DVE add)

All matmul operands bf16 (PE full rate); PSUM accumulation fp32; softmax
sum in fp32 via activation accum_out.  Weights are pre-transposed/cast on
host and replicated to all 8 cores.

PSUM budget (8 banks): S/exp pipeline 2 slots x [128,1024] = 4 banks;
P2 accumulator [128,1024] = 2 banks; all other psums ([128,512] = 1 bank)
rotate through a 2-slot pool = 2 banks.
"""

import numpy as np
import ml_dtypes

import concourse.bass as bass
import concourse.mybir as mybir
import concourse.tile as tile
from concourse.bass_utils import run_bass_kernel_spmd

BF = mybir.dt.bfloat16
F32 = mybir.dt.float32
EXP = mybir.ActivationFunctionType.Exp

B, C, HH, WW = 32, 64, 32, 32
N = HH * WW          # 1024
NCORES = 8
BPC = B // NCORES    # 4 batch items per core
NPAIRS = BPC // 2    # 2 stacked pairs per core
NK = N // 128        # 8 chunks of 128 along the N dimension
NH = 512             # matmul free-dim half (one PSUM bank)


def _build_body(nc, consts, acts, bigacts, psAcc, psS, psSm,
                x32, x16, wthT, wphT, wgvT, wmaT, wmkT, out_e):
    # ---- load weights once ----
    wth = consts.tile([128, C], BF, tag="wth")
    wph = consts.tile([128, C], BF, tag="wph")
    wgv = consts.tile([128, C], BF, tag="wgv")
    wma = consts.tile([128, C], BF, tag="wma")
    nc.sync.dma_start(wth[:], wthT[:])
    nc.sync.dma_start(wph[:], wphT[:])
    nc.sync.dma_start(wgv[:], wgvT[:])
    nc.sync.dma_start(wma[:], wmaT[:])
    wmk = []
    for m in range(NK):
        t = consts.tile([128, N], BF, tag=f"wmk{m}")
        nc.sync.dma_start(t[:], wmkT[m * 128:(m + 1) * 128, :])
        wmk.append(t)

    lo = slice(0, 64)
    hi = slice(64, 128)

    for p in range(NPAIRS):
        rows = slice(p * 128, (p + 1) * 128)
        xb = acts.tile([128, N], BF, tag="xb")
        xf = acts.tile([128, N], F32, tag="xf")
        nc.sync.dma_start(xb[:], x16[rows, :])
        nc.sync.dma_start(xf[:], x32[rows, :])

        # ---- stage 1: PhiT (row-split), T (diag quadrants), GT (row-split)
        psPhiT_b = psSm.tile([128, NH], F32, tag="psSm")
        psPhiT_c = psSm.tile([128, NH], F32, tag="psSm")
        for m in range(NK):
            mm = slice(m * 128, (m + 1) * 128)
            cc = slice(m * C, (m + 1) * C)
            nc.tensor.matmul(psPhiT_b[:, cc], lhsT=xb[lo, mm], rhs=wph[lo, :])
            nc.tensor.matmul(psPhiT_c[:, cc], lhsT=xb[hi, mm], rhs=wph[hi, :])
        PhiT_b = acts.tile([128, NH], BF, tag="PhiT_b")
        PhiT_c = acts.tile([128, NH], BF, tag="PhiT_c")
        nc.vector.tensor_copy(out=PhiT_b[:], in_=psPhiT_b[:])
        nc.vector.tensor_copy(out=PhiT_c[:], in_=psPhiT_c[:])

        T_sb = acts.tile([128, N], BF, tag="T_sb")
        for h in range(2):
            hh = slice(h * NH, (h + 1) * NH)
            psT = psSm.tile([128, NH], F32, tag="psSm")
            nc.tensor.matmul(psT[lo, :], lhsT=wth[lo, :], rhs=xb[lo, hh])
            nc.tensor.matmul(psT[hi, :], lhsT=wth[hi, :], rhs=xb[hi, hh])
            nc.vector.tensor_copy(out=T_sb[:, hh], in_=psT[:])

        psGT_b = psSm.tile([128, NH], F32, tag="psSm")
        psGT_c = psSm.tile([128, NH], F32, tag="psSm")
        for m in range(NK):
            mm = slice(m * 128, (m + 1) * 128)
            cc = slice(m * C, (m + 1) * C)
            nc.tensor.matmul(psGT_b[:, cc], lhsT=xb[lo, mm], rhs=wgv[lo, :])
            nc.tensor.matmul(psGT_c[:, cc], lhsT=xb[hi, mm], rhs=wgv[hi, :])
        GT_b = acts.tile([128, NH], BF, tag="GT_b")
        GT_c = acts.tile([128, NH], BF, tag="GT_c")
        nc.vector.tensor_copy(out=GT_b[:], in_=psGT_b[:])
        nc.vector.tensor_copy(out=GT_c[:], in_=psGT_c[:])

        # ---- P2 = PhiT^T @ wmkT   [c, k] col-split by batch, accum over m
        psP2 = psAcc.tile([128, N], F32, tag="psP2")
        for m in range(NK):
            cc = slice(m * C, (m + 1) * C)
            for h in range(2):
                hh = slice(h * NH, (h + 1) * NH)
                nc.tensor.matmul(psP2[lo, hh], lhsT=PhiT_b[:, cc],
                                 rhs=wmk[m][:, hh],
                                 start=(m == 0), stop=(m == NK - 1))
                nc.tensor.matmul(psP2[hi, hh], lhsT=PhiT_c[:, cc],
                                 rhs=wmk[m][:, hh],
                                 start=(m == 0), stop=(m == NK - 1))
        P2 = acts.tile([128, N], BF, tag="P2")
        nc.vector.tensor_copy(out=P2[:], in_=psP2[:])

        # ---- S = P2^T @ T per k-chunk (row-split by batch), exp + row-sum
        E_b = bigacts.tile([128, NK, N], BF, tag="E_b")
        E_c = bigacts.tile([128, NK, N], BF, tag="E_c")
        D_b = acts.tile([128, NK], F32, tag="D_b")
        D_c = acts.tile([128, NK], F32, tag="D_c")
        for k in range(NK):
            kk = slice(k * 128, (k + 1) * 128)
            psS_b = psS.tile([128, N], F32, tag="psS")
            for h in range(2):
                hh = slice(h * NH, (h + 1) * NH)
                nc.tensor.matmul(psS_b[:, hh], lhsT=P2[lo, kk], rhs=T_sb[lo, hh])
            nc.scalar.activation(E_b[:, k, :], psS_b[:], EXP,
                                 accum_out=D_b[:, k:k + 1])
            psS_c = psS.tile([128, N], F32, tag="psS")
            for h in range(2):
                hh = slice(h * NH, (h + 1) * NH)
                nc.tensor.matmul(psS_c[:, hh], lhsT=P2[hi, kk], rhs=T_sb[hi, hh])
            nc.scalar.activation(E_c[:, k, :], psS_c[:], EXP,
                                 accum_out=D_c[:, k:k + 1])

        # ---- fold 1/D into GT
        R_b = acts.tile([128, NK], F32, tag="R_b")
        R_c = acts.tile([128, NK], F32, tag="R_c")
        nc.vector.reciprocal(R_b[:], D_b[:])
        nc.vector.reciprocal(R_c[:], D_c[:])
        GTs_b = acts.tile([128, NH], BF, tag="GTs_b")
        GTs_c = acts.tile([128, NH], BF, tag="GTs_c")
        for m in range(NK):
            cc = slice(m * C, (m + 1) * C)
            nc.vector.tensor_scalar_mul(GTs_b[:, cc], GT_b[:, cc], R_b[:, m:m + 1])
            nc.vector.tensor_scalar_mul(GTs_c[:, cc], GT_c[:, cc], R_c[:, m:m + 1])

        # ---- O = GTs^T @ E  col-split by batch, accum over m (per half)
        O_sb = acts.tile([128, N], BF, tag="O_sb")
        for h in range(2):
            hh = slice(h * NH, (h + 1) * NH)
            psO = psSm.tile([128, NH], F32, tag="psSm")
            for m in range(NK):
                cc = slice(m * C, (m + 1) * C)
                nc.tensor.matmul(psO[lo, :], lhsT=GTs_b[:, cc],
                                 rhs=E_b[:, m, hh],
                                 start=(m == 0), stop=(m == NK - 1))
                nc.tensor.matmul(psO[hi, :], lhsT=GTs_c[:, cc],
                                 rhs=E_c[:, m, hh],
                                 start=(m == 0), stop=(m == NK - 1))
            nc.vector.tensor_copy(out=O_sb[:, hh], in_=psO[:])

        # ---- mask (diag quadrants) + residual add, DMA out
        out_sb = acts.tile([128, N], F32, tag="out_sb")
        for h in range(2):
            hh = slice(h * NH, (h + 1) * NH)
            psM = psSm.tile([128, NH], F32, tag="psSm")
            nc.tensor.matmul(psM[lo, :], lhsT=wma[lo, :], rhs=O_sb[lo, hh])
            nc.tensor.matmul(psM[hi, :], lhsT=wma[hi, :], rhs=O_sb[hi, hh])
            nc.vector.tensor_tensor(out_sb[:, hh], psM[:], xf[:, hh],
                                    mybir.AluOpType.add)
        nc.sync.dma_start(out_e[p * 128:(p + 1) * 128, :], out_sb[:])


_SPLIT_WAIT_TYPES = {
    "InstMatmult", "InstTensorTensor", "InstTensorCopy", "InstActivation",
    "InstTensorScalarPtr", "InstTensorScalar", "InstReciprocal",
    "InstTensorReduce", "InstMemSet", "InstLdweights", "InstTranspose",
    "InstTensorTensorScan", "InstSelect", "InstCopy", "InstDMACopy",
    "InstTensorLoad", "InstTensorSave", "InstDrain",
}


def _split_matmul_waits(nc):
    """Walrus's TRN2 codegen allows at most one sync-wait per compute
    instruction (MM/TT/... ISA structs carry a single wait slot).

    Tile emits 2-3 (PSUM slot WAW + operand producers).  Hoist every wait
    of a multi-wait compute instruction onto NoOps placed right before it
    on the same engine — the NX sequencer executes them in order, so
    semantics are identical.  DMA instructions are left untouched.
    """
    cnt = 0
    for blk in nc.m.functions[0].blocks:
        insts = blk.instructions
        new = []
        for ins in insts:
            si = getattr(ins, "sync_info", None)
            if (type(ins).__name__ in _SPLIT_WAIT_TYPES and si is not None
                    and si.on_wait and len(si.on_wait) > 1):
                for j, w in enumerate(si.on_wait):
                    nop = mybir.InstNoOp(
                        name=f"{ins.name}-w{j}",
                        engine=ins.engine,
                        sync_info=mybir.SyncInfo(on_wait=[w], on_update=[]),
                        bass_nofuse=True,
                    )
                    new.append(nop)
                ins.sync_info = mybir.SyncInfo(
                    on_wait=[], on_update=list(si.on_update))
                cnt += 1
            new.append(ins)
        blk.instructions = new
    return cnt


def build_nc_full():
    nc = bass.Bass()
    # Per-core inputs.  x rows: pair p occupies partitions [0:128) as
    # (batch 2p on 0-63, batch 2p+1 on 64-127) after slicing [p*128:(p+1)*128).
    x32 = nc.declare_dram_parameter("x32", [BPC * C, N], F32, isOutput=False)
    x16 = nc.declare_dram_parameter("x16", [BPC * C, N], BF, isOutput=False)
    wthT = nc.declare_dram_parameter("wthT", [128, C], BF, isOutput=False)
    wphT = nc.declare_dram_parameter("wphT", [128, C], BF, isOutput=False)
    wgvT = nc.declare_dram_parameter("wgvT", [128, C], BF, isOutput=False)
    wmaT = nc.declare_dram_parameter("wmaT", [128, C], BF, isOutput=False)
    wmkT = nc.declare_dram_parameter("wmkT", [N, N], BF, isOutput=False)
    out_e = nc.declare_dram_parameter("out", [BPC * C, N], F32, isOutput=True)

    with tile.TileContext(nc) as tc:
        with (
            tc.tile_pool(name="consts", bufs=1) as consts,
            tc.tile_pool(name="acts", bufs=2) as acts,
            tc.tile_pool(name="bigacts", bufs=2) as bigacts,
            tc.tile_pool(name="psAcc", bufs=1, space="PSUM") as psAcc,
            tc.tile_pool(name="psS", bufs=2, space="PSUM") as psS,
            tc.tile_pool(name="psSm", bufs=2, space="PSUM") as psSm,
        ):
            _build_body(nc, consts, acts, bigacts, psAcc, psS, psSm,
                        x32, x16, wthT, wphT, wgvT, wmaT, wmkT, out_e)
    _split_matmul_waits(nc)
    return nc


def _prep_weights(w_phi, w_theta, w_g, w_mask, w_mv, w_mk):
    bf = ml_dtypes.bfloat16

    def dup(a):  # [64, 64] -> [128, 64], duplicated on both partition halves
        return np.ascontiguousarray(np.concatenate([a, a], axis=0)).astype(bf)

    w_gv = (w_mv.astype(np.float64) @ w_g.astype(np.float64)).astype(np.float32)
    return {
        "wthT": dup(w_theta.T),
        "wphT": dup(w_phi.T),
        "wgvT": dup(w_gv.T),
        "wmaT": dup(w_mask.T),
        "wmkT": np.ascontiguousarray(w_mk.T).astype(bf),
    }


def kernel(x, w_phi, w_theta, w_g, w_mask, w_mv, w_mk, _trace=False):
    bf = ml_dtypes.bfloat16
    x = np.asarray(x, dtype=np.float32)
    weights = _prep_weights(np.asarray(w_phi, np.float32),
                            np.asarray(w_theta, np.float32),
                            np.asarray(w_g, np.float32),
                            np.asarray(w_mask, np.float32),
                            np.asarray(w_mv, np.float32),
                            np.asarray(w_mk, np.float32))

    xr = x.reshape(B, C, N)
    in_maps = []
    for i in range(NCORES):
        shard = np.ascontiguousarray(xr[i * BPC:(i + 1) * BPC]).reshape(BPC * C, N)
        m = {"x32": shard, "x16": shard.astype(bf)}
        m.update(weights)
        in_maps.append(m)

    nc = build_nc_full()
    res = run_bass_kernel_spmd(nc, in_maps, list(range(NCORES)), trace=_trace)
    outs = [np.asarray(res.results[i]["out"]).reshape(BPC, C, HH, WW)
            for i in range(NCORES)]
    full = np.concatenate(outs, axis=0)
    if _trace:
        return full, res
    return full
